# revision 1
# baseline (speedup 1.0000x reference)
"""CaptionDecoder Trainium2 kernel: 8-core SPMD.

Strategy:
  - Recurrence (attention + LSTM, T=32 steps) is batch-sharded: each core
    owns 4 of 32 batch rows; all weights replicated. No collectives.
  - Hidden states for all steps are then AllGathered (2MB, one collective)
    and the big vocab projection [1024,512]@[512,32000] is vocab-sharded:
    each core computes a 4000-wide vocab slice for the full batch.
  - All matmul operands are bf16 (fp32 PSUM accumulation); state c and
    softmax statistics stay fp32. Validated vs fp32 reference in numpy:
    rel_fro ~3.7e-3.
Layouts are "transposed": feature dims on SBUF partitions, batch on the
matmul moving/free dim, so LSTM pointwise ops run on 128 active lanes.
"""
import sys
import numpy as np
import ml_dtypes

sys.path.insert(0, "/opt/trn_rl_repo")

from contextlib import ExitStack

import concourse.bass as bass
import concourse.tile as tile
from concourse import bacc, mybir
from concourse.bass_utils import run_bass_kernel_spmd

BF16 = mybir.dt.bfloat16
F32 = mybir.dt.float32
AF = mybir.ActivationFunctionType
ALU = mybir.AluOpType

E = 512
H = 512
V = 32000
B = 32
P = 196
T = 32
N_CORES = 8
BL = B // N_CORES          # 4 batch rows per core
VS = V // N_CORES          # 4000 vocab per core
KH = 4                     # 128-chunks of E / H
GH = 16                    # 128-chunks of 4H
PC0, PC1 = 128, P - 128    # pixel chunks 128 + 68

bf16 = ml_dtypes.bfloat16


def _to_tiles(mat_T):
    """[K, M] -> [128, K//128, M] (partition-major K tiles)."""
    Kdim, M = mat_T.shape
    return mat_T.reshape(Kdim // 128, 128, M).transpose(1, 0, 2)


def build_nc(n_cores):
    nc = bacc.Bacc(
        "TRN2",
        target_bir_lowering=False,
        debug=False,
        enable_asserts=False,
        num_devices=n_cores,
    )

    def inp(name, shape, dt=BF16):
        return nc.declare_dram_parameter(name, list(shape), dt, isOutput=False).ap()

    # Per-core sharded inputs
    featT_p = inp("featT", [128, KH, BL, P])            # features^T [el, eh, b, p]
    feat_p = inp("feat", [128, BL, 2, E])               # [p_lo, b, pc, e] (pc1 rows>=68 pad)
    embT_p = inp("embT", [128, KH, BL * T])             # gathered emb^T [el, eh, (b,t)]
    linWT_p = inp("linWT", [128, KH, VS])               # lin_W shard^T
    linb_p = inp("linb", [128, VS], F32)                # host-expanded over partitions
    # Replicated weights
    WfT_p = inp("WfT", [128, KH, H])
    WhT_p = inp("WhT", [128, KH, H])
    WcombT_p = inp("WcombT", [128, 2 * KH, 4 * H])      # [ctx;h] -> gates
    WembT_p = inp("WembT", [128, KH, 4 * H])
    ihWT_p = inp("ihWT", [128, KH, H])                  # inith_W^T / P
    icWT_p = inp("icWT", [128, KH, H])
    ihb_p = inp("ihb", [128, KH, BL], F32)
    icb_p = inp("icb", [128, KH, BL], F32)
    attnb_p = inp("attnb", [128, KH, BL], F32)
    bcomb_p = inp("bcomb", [128, GH, BL * T], F32)      # (b_ih+b_hh) expanded
    vmaskT_p = inp("vmaskT", [128, KH, BL, BL])         # v masked per batch col
    eye4_p = inp("eye4", [4, 4])

    out_p = nc.declare_dram_parameter(
        "out", [n_cores * BL * T, VS], F32, isOutput=True
    ).ap()

    hbounce = nc.dram_tensor("hbounce", [128, KH * BL * T], BF16).ap()
    hgath = nc.dram_tensor(
        "hgath", [n_cores * 128, KH * BL * T], BF16, addr_space="Shared"
    ).ap()

    with tile.TileContext(nc) as tc, ExitStack() as ctx:
        const = ctx.enter_context(tc.tile_pool(name="const", bufs=1))
        state = ctx.enter_context(tc.tile_pool(name="state", bufs=1))
        work = ctx.enter_context(tc.tile_pool(name="work", bufs=2))

        # ---- persistent SBUF ----
        feat_sb = const.tile([128, BL, 2, E], BF16)
        WhT_sb = const.tile([128, KH, H], BF16)
        WcombT_sb = const.tile([128, 2 * KH, 4 * H], BF16)
        vmaskT_sb = const.tile([128, KH, BL, BL], BF16)
        eye4_sb = const.tile([4, 4], BF16)
        attnb_sb = const.tile([128, KH, BL], F32)
        featproT = const.tile([128, KH, BL, P], BF16)   # Wf@features^T (+0)
        embprojT = const.tile([128, GH, BL * T], F32)   # Wemb@emb^T + b_ih + b_hh
        linWT_sb = const.tile([128, KH, VS], BF16)
        linb_sb = const.tile([128, VS], F32)

        ctxhT = state.tile([128, 2 * KH, BL], BF16)     # [ctx^T; h^T] gate input
        cT = state.tile([128, KH, BL], F32)
        h_histT = state.tile([128, KH, BL, T], F32)
        alphaT_mask = state.tile([128, 2, BL, BL], BF16)

        nc.sync.dma_start(feat_sb[:], feat_p[:])
        nc.sync.dma_start(WhT_sb[:], WhT_p[:])
        nc.sync.dma_start(WcombT_sb[:], WcombT_p[:])
        nc.sync.dma_start(vmaskT_sb[:], vmaskT_p[:])
        nc.sync.dma_start(eye4_sb[:], eye4_p[:])
        nc.sync.dma_start(attnb_sb[:], attnb_p[:])
        nc.sync.dma_start(linWT_sb[:], linWT_p[:])
        nc.sync.dma_start(linb_sb[:], linb_p[:])
        nc.gpsimd.memset(alphaT_mask[:], 0.0)

        # ---- precompute ----
        with (
            tc.tile_pool(name="pre", bufs=1) as pre,
            tc.tile_pool(name="prepsum", bufs=1, space="PSUM") as prepsum,
        ):
            featT_sb = pre.tile([128, KH, BL, P], BF16)
            embT_sb = pre.tile([128, KH, BL * T], BF16)
            WfT_sb = pre.tile([128, KH, H], BF16)
            WembT_sb = pre.tile([128, KH, 4 * H], BF16)
            ihWT_sb = pre.tile([128, KH, H], BF16)
            icWT_sb = pre.tile([128, KH, H], BF16)
            ihb_sb = pre.tile([128, KH, BL], F32)
            icb_sb = pre.tile([128, KH, BL], F32)
            bcomb_sb = pre.tile([128, GH, BL * T], F32)
            meanT_bf = pre.tile([128, KH, BL], BF16)

            nc.sync.dma_start(featT_sb[:], featT_p[:])
            nc.sync.dma_start(embT_sb[:], embT_p[:])
            nc.sync.dma_start(WfT_sb[:], WfT_p[:])
            nc.sync.dma_start(WembT_sb[:], WembT_p[:])
            nc.sync.dma_start(ihWT_sb[:], ihWT_p[:])
            nc.sync.dma_start(icWT_sb[:], icWT_p[:])
            nc.sync.dma_start(ihb_sb[:], ihb_p[:])
            nc.sync.dma_start(icb_sb[:], icb_p[:])
            nc.sync.dma_start(bcomb_sb[:], bcomb_p[:])

            meanT_f = pre.tile([128, KH, BL], F32)
            nc.vector.tensor_reduce(
                meanT_f[:], featT_sb[:], axis=mybir.AxisListType.X, op=ALU.add
            )
            nc.vector.tensor_copy(meanT_bf[:], meanT_f[:])

            # h0 / c0 (1/P folded into ihWT/icWT host-side)
            psum_h0 = prepsum.tile([128, KH, BL], F32)
            psum_c0 = prepsum.tile([128, KH, BL], F32)
            for mh in range(KH):
                for kh in range(KH):
                    nc.tensor.matmul(
                        psum_h0[:, mh, :],
                        ihWT_sb[:, kh, mh * 128:(mh + 1) * 128],
                        meanT_bf[:, kh, :],
                        start=(kh == 0), stop=(kh == KH - 1),
                    )
            for mh in range(KH):
                for kh in range(KH):
                    nc.tensor.matmul(
                        psum_c0[:, mh, :],
                        icWT_sb[:, kh, mh * 128:(mh + 1) * 128],
                        meanT_bf[:, kh, :],
                        start=(kh == 0), stop=(kh == KH - 1),
                    )
            nc.vector.tensor_add(ctxhT[:, KH:2 * KH, :], psum_h0[:], ihb_sb[:])
            nc.vector.tensor_add(cT[:], psum_c0[:], icb_sb[:])

            # featproT = Wf @ features^T
            for mh in range(KH):
                for half in range(2):
                    psum_fp = prepsum.tile([128, 2, P], F32)
                    for kh in range(KH):
                        nc.tensor.matmul(
                            psum_fp[:],
                            WfT_sb[:, kh, mh * 128:(mh + 1) * 128],
                            featT_sb[:, kh, 2 * half:2 * half + 2, :],
                            start=(kh == 0), stop=(kh == KH - 1),
                        )
                    nc.vector.tensor_copy(
                        featproT[:, mh, 2 * half:2 * half + 2, :], psum_fp[:]
                    )

            # embprojT = Wemb @ emb^T + (b_ih + b_hh)
            for mh in range(GH):
                psum_ep = prepsum.tile([128, BL * T], F32)
                for kh in range(KH):
                    nc.tensor.matmul(
                        psum_ep[:],
                        WembT_sb[:, kh, mh * 128:(mh + 1) * 128],
                        embT_sb[:, kh, :],
                        start=(kh == 0), stop=(kh == KH - 1),
                    )
                nc.vector.tensor_add(
                    embprojT[:, mh, :], psum_ep[:], bcomb_sb[:, mh, :]
                )

        # ---- recurrence ----
        psum = ctx.enter_context(tc.tile_pool(name="psum", bufs=1, space="PSUM"))
        for t in range(T):
            # hWh^T [h_out, b] — head of the per-step critical chain
            psum_hwh = psum.tile([128, KH, BL], F32)
            for mh in range(KH):
                for kh in range(KH):
                    nc.tensor.matmul(
                        psum_hwh[:, mh, :],
                        WhT_sb[:, kh, mh * 128:(mh + 1) * 128],
                        ctxhT[:, KH + kh, :],
                        start=(kh == 0), stop=(kh == KH - 1),
                    )
            bias_bf = work.tile([128, KH, BL], BF16)
            nc.vector.tensor_add(bias_bf[:], psum_hwh[:], attnb_sb[:])

            # gates h-part: off-chain, fills PE idle during energy/tanh
            psum_gh = psum.tile([128, GH, BL], F32)
            for mh in range(GH):
                for kh in range(KH, 2 * KH):
                    nc.tensor.matmul(
                        psum_gh[:, mh, :],
                        WcombT_sb[:, kh, mh * 128:(mh + 1) * 128],
                        ctxhT[:, kh, :],
                        start=(kh == KH), stop=(kh == 2 * KH - 1),
                    )

            # energy = tanh(featproT + bias); scores = v . energy
            # per-chunk tiles so TT(hh+1) overlaps Tanh(hh)
            psum_sc = psum.tile([4, P], F32, tag="scctx")
            for hh in range(KH):
                energy = work.tile([128, BL, P], BF16, tag=f"en{hh}")
                energy_t = work.tile([128, BL, P], BF16, tag=f"et{hh}")
                nc.vector.tensor_add(
                    energy[:],
                    featproT[:, hh, :, :],
                    bias_bf[:, hh, :].unsqueeze(2).broadcast_to([128, BL, P]),
                )
                nc.scalar.activation(energy_t[:], energy[:], AF.Tanh)
                for b in range(BL):
                    nc.tensor.matmul(
                        psum_sc[0:4, :],
                        vmaskT_sb[:, hh, b, :],
                        energy_t[:, b, :],
                        start=(hh == 0 and b == 0),
                        stop=(hh == KH - 1 and b == BL - 1),
                    )

            # softmax over p. exp via e^s=(1+tanh(s/2))/(1-tanh(s/2)) to stay
            # on the sigmoid/tanh ACT table set (an Exp call would force two
            # ~2.7us table reloads per step). Scores are small; no max-sub.
            th = work.tile([4, P], F32)
            num = work.tile([4, P], F32)
            den = work.tile([4, P], F32)
            rden = work.tile([4, P], F32)
            esc = work.tile([4, P], F32)
            esum = work.tile([4, 1], F32)
            rsum = work.tile([4, 1], F32)
            alpha = work.tile([4, P], BF16)
            nc.scalar.activation(th[0:4, :], psum_sc[0:4, :], AF.Tanh, scale=0.5)
            nc.vector.tensor_scalar_add(num[0:4, :], th[0:4, :], 1.0)
            nc.vector.tensor_scalar(den[0:4, :], th[0:4, :], -1.0, 1.0,
                                    op0=ALU.mult, op1=ALU.add)
            nc.vector.reciprocal(rden[0:4, :], den[0:4, :])
            nc.vector.tensor_mul(esc[0:4, :], num[0:4, :], rden[0:4, :])
            nc.vector.reduce_sum(esum[0:4, :], esc[0:4, :],
                                 axis=mybir.AxisListType.X)
            nc.vector.reciprocal(rsum[0:4, :], esum[0:4, :])
            nc.vector.tensor_scalar_mul(alpha[0:4, :], esc[0:4, :], rsum[0:4, :])

            # alpha^T (per-b masked columns; off-diagonal stays zero forever)
            psum_aT = psum.tile([128, 2, BL], BF16, tag="tpose")
            nc.tensor.transpose(psum_aT[:, 0, :], alpha[0:4, 0:PC0], eye4_sb[:])
            nc.tensor.transpose(psum_aT[0:PC1, 1, :], alpha[0:4, PC0:P], eye4_sb[:])
            for b in range(BL):
                nc.vector.tensor_copy(
                    alphaT_mask[:, 0, b, b:b + 1], psum_aT[:, 0, b:b + 1]
                )
                nc.vector.tensor_copy(
                    alphaT_mask[0:PC1, 1, b, b:b + 1], psum_aT[0:PC1, 1, b:b + 1]
                )

            # context [b, e] then ctx^T into gate input
            psum_ctx = psum.tile([4, E], F32, tag="scctx")
            first = True
            for b in range(BL):
                for pc in range(2):
                    kk = PC0 if pc == 0 else PC1
                    nc.tensor.matmul(
                        psum_ctx[0:4, :],
                        alphaT_mask[0:kk, pc, b, :],
                        feat_sb[0:kk, b, pc, :],
                        start=first, stop=(b == BL - 1 and pc == 1),
                    )
                    first = False
            ctx_sb = work.tile([4, E], BF16)
            nc.vector.tensor_copy(ctx_sb[0:4, :], psum_ctx[0:4, :])
            psum_ctxT = psum.tile([128, KH, BL], BF16, tag="tpose")
            for kh in range(KH):
                nc.tensor.transpose(
                    psum_ctxT[:, kh, :], ctx_sb[0:4, kh * 128:(kh + 1) * 128],
                    eye4_sb[:],
                )
            nc.vector.tensor_copy(ctxhT[:, 0:KH, :], psum_ctxT[:])

            # gates ctx-part (h-part already accumulating in psum_gh)
            psum_g = psum.tile([128, GH, BL], F32)
            for mh in range(GH):
                for kh in range(KH):
                    nc.tensor.matmul(
                        psum_g[:, mh, :],
                        WcombT_sb[:, kh, mh * 128:(mh + 1) * 128],
                        ctxhT[:, kh, :],
                        start=(kh == 0), stop=(kh == KH - 1),
                    )
            gates_hb = work.tile([128, GH, BL], F32)
            nc.vector.tensor_add(gates_hb[:], psum_gh[:], embprojT[:, :, t::T])
            gates_sb = work.tile([128, GH, BL], F32)
            nc.vector.tensor_add(gates_sb[:], psum_g[:], gates_hb[:])

            # LSTM pointwise (gate chunks: i=0:4, f=4:8, g=8:12, o=12:16)
            sig_if = work.tile([128, 8, BL], F32)
            tanh_g = work.tile([128, KH, BL], F32)
            sig_o = work.tile([128, KH, BL], F32)
            nc.scalar.activation(sig_if[:], gates_sb[:, 0:8, :], AF.Sigmoid)
            nc.scalar.activation(tanh_g[:], gates_sb[:, 8:12, :], AF.Tanh)
            nc.scalar.activation(sig_o[:], gates_sb[:, 12:16, :], AF.Sigmoid)
            t1 = work.tile([128, KH, BL], F32)
            t2 = work.tile([128, KH, BL], F32)
            nc.vector.tensor_mul(t1[:], sig_if[:, 4:8, :], cT[:])
            nc.vector.tensor_mul(t2[:], sig_if[:, 0:4, :], tanh_g[:])
            nc.vector.tensor_add(cT[:], t1[:], t2[:])
            tanh_c = work.tile([128, KH, BL], F32)
            nc.scalar.activation(tanh_c[:], cT[:], AF.Tanh)
            nc.vector.tensor_mul(h_histT[:, :, :, t], sig_o[:], tanh_c[:])
            nc.vector.tensor_copy(ctxhT[:, KH:2 * KH, :], h_histT[:, :, :, t])

        # ---- phase 2: gather h, vocab-sharded projection ----
        with (
            tc.tile_pool(name="ph2", bufs=2) as ph2,
            tc.tile_pool(name="ph2psum", bufs=2, space="PSUM") as ph2psum,
        ):
            hb_sb = ph2.tile([128, KH * BL * T], BF16, bufs=1)
            nc.vector.tensor_copy(hb_sb[:], h_histT[:])
            if n_cores > 1:
                nc.sync.dma_start(hbounce[:], hb_sb[:])
                nc.gpsimd.collective_compute(
                    "AllGather",
                    ALU.bypass,
                    replica_groups=[list(range(n_cores))],
                    ins=[hbounce[:]],
                    outs=[hgath[:]],
                )
            NCH = VS // 500
            for r in range(n_cores):
                if n_cores > 1:
                    hall = ph2.tile([128, KH * BL * T], BF16)
                    nc.sync.dma_start(
                        hall[:], hgath[r * 128:(r + 1) * 128, :]
                    )
                else:
                    hall = hb_sb
                hall_v = hall.rearrange("p (kh m) -> p kh m", kh=KH)
                out_sb = ph2.tile([128, VS], F32)
                for nch in range(NCH):
                    psum_o = ph2psum.tile([128, 500], F32)
                    for kh in range(KH):
                        nc.tensor.matmul(
                            psum_o[:],
                            hall_v[:, kh, :],
                            linWT_sb[:, kh, nch * 500:(nch + 1) * 500],
                            start=(kh == 0), stop=(kh == KH - 1),
                        )
                    nc.vector.tensor_add(
                        out_sb[:, nch * 500:(nch + 1) * 500],
                        psum_o[:],
                        linb_sb[:, nch * 500:(nch + 1) * 500],
                    )
                nc.sync.dma_start(out_p[r * 128:(r + 1) * 128, :], out_sb[:])

    nc.compile()
    return nc


def make_in_maps(inputs, n_cores):
    f32 = np.float32
    feats = np.asarray(inputs["features"], f32)          # [B, P, E]
    caps = np.asarray(inputs["captions"]).astype(np.int64)
    embW = np.asarray(inputs["embed_W"], f32)
    attnW = np.asarray(inputs["attn_W"], f32)
    attnb = np.asarray(inputs["attn_b"], f32)
    vw = np.asarray(inputs["v_w"], f32)
    Wih = np.asarray(inputs["W_ih"], f32)
    Whh = np.asarray(inputs["W_hh"], f32)
    bih = np.asarray(inputs["b_ih"], f32)
    bhh = np.asarray(inputs["b_hh"], f32)
    linW = np.asarray(inputs["lin_W"], f32)
    linb = np.asarray(inputs["lin_b"], f32)
    ihW = np.asarray(inputs["inith_W"], f32)
    ihb = np.asarray(inputs["inith_b"], f32)
    icW = np.asarray(inputs["initc_W"], f32)
    icb = np.asarray(inputs["initc_b"], f32)

    Wf, Wh = attnW[:, :E], attnW[:, E:]
    Wemb, Wctx = Wih[:, :E], Wih[:, E:]
    Wcomb = np.concatenate([Wctx, Whh], axis=1)          # [4H, E+H]

    def bft(m):  # [K, M] fp32 -> [128, K//128, M] bf16 tiles
        return np.ascontiguousarray(_to_tiles(m)).astype(bf16)

    WfT_h = bft(Wf.T)
    WhT_h = bft(Wh.T)
    WcombT_h = bft(Wcomb.T)
    WembT_h = bft(Wemb.T)
    ihWT_h = bft(ihW.T / P)
    icWT_h = bft(icW.T / P)

    def pexp(vec, reps):  # [D] -> [128, D//128, reps] f32
        return np.repeat(
            vec.reshape(-1, 128).T[:, :, None], reps, axis=2
        ).astype(f32)

    ihb_h = pexp(ihb, BL)
    icb_h = pexp(icb, BL)
    attnb_h = pexp(attnb, BL)
    bcomb_h = pexp(bih + bhh, BL * T)
    linb_full = linb
    eye4_h = np.eye(4, dtype=bf16)

    vmask = np.zeros((128, KH, BL, BL), np.float32)
    vt = vw.reshape(KH, 128).T                            # [128, KH]
    for b in range(BL):
        vmask[:, :, b, b] = vt
    vmask_h = vmask.astype(bf16)

    in_maps = []
    for k in range(n_cores):
        b0 = k * BL
        fk = feats[b0:b0 + BL]                            # [BL, P, E]
        featT = (
            fk.transpose(2, 0, 1)
            .reshape(KH, 128, BL, P)
            .transpose(1, 0, 2, 3)
        )
        featpad = np.zeros((BL, 2, 128, E), f32)
        featpad[:, 0] = fk[:, 0:128]
        featpad[:, 1, 0:PC1] = fk[:, 128:P]
        feat_h = featpad.transpose(2, 0, 1, 3)            # [128, BL, 2, E]
        embk = embW[caps[b0:b0 + BL]]                     # [BL, T, E]
        embT = (
            embk.transpose(2, 0, 1)
            .reshape(KH, 128, BL * T)
            .transpose(1, 0, 2)
        )
        linWT_k = _to_tiles(linW[k * VS:(k + 1) * VS].T)  # [128, KH, VS]
        linb_k = np.repeat(
            linb_full[k * VS:(k + 1) * VS][None, :], 128, axis=0
        ).astype(f32)
        in_maps.append({
            "featT": np.ascontiguousarray(featT).astype(bf16),
            "feat": np.ascontiguousarray(feat_h).astype(bf16),
            "embT": np.ascontiguousarray(embT).astype(bf16),
            "linWT": np.ascontiguousarray(linWT_k).astype(bf16),
            "linb": linb_k,
            "WfT": WfT_h, "WhT": WhT_h, "WcombT": WcombT_h, "WembT": WembT_h,
            "ihWT": ihWT_h, "icWT": icWT_h,
            "ihb": ihb_h, "icb": icb_h, "attnb": attnb_h, "bcomb": bcomb_h,
            "vmaskT": vmask_h, "eye4": eye4_h,
        })
    return in_maps


def unshard(results, n_cores):
    # each core's "out": [n_cores*BL*T, VS] rows ordered (rank, b_local, t)
    shards = [
        np.asarray(results[k]["out"]).reshape(n_cores * BL, T, VS)
        for k in range(n_cores)
    ]
    return np.concatenate(shards, axis=-1).reshape(B, T, V).astype(np.float32)


_NC_CACHE = {}


def kernel(**inputs):
    n_cores = N_CORES
    if n_cores not in _NC_CACHE:
        _NC_CACHE[n_cores] = build_nc(n_cores)
    nc = _NC_CACHE[n_cores]
    in_maps = make_in_maps(inputs, n_cores)
    res = run_bass_kernel_spmd(nc, in_maps, list(range(n_cores)))
    return unshard(res.results, n_cores)


if __name__ == "__main__":
    import reference
    inputs = reference.setup_inputs()
    out = kernel(**{k: np.asarray(v) for k, v in inputs.items()})
    print(out.shape, out.dtype)

